# revision 3
# baseline (speedup 1.0000x reference)
"""Trainium2 Bass kernel for nn_CategoryAlign_Module (pooling / cross Pearson).

Math (see reference):
  for each stream s in {1,2}:
    vec_b[k,c]  = sum_p preds[b,k,p] * feats[b,c,p] / sum_p preds[b,k,p]
    ctx_b[k,c]  = vec_b[k,c] / max(||vec_b[:,c]||_2, 1e-12)      (norm over K)
    ctx[k,c]    = mean_b ctx_b[k,c]
  out = pearson(ctx1, ctx2)   (center+normalize rows over C, then ctx1 @ ctx2^T)

Distribution: data-parallel over the batch dim, one batch element per
NeuronCore (B=8, 8 cores).  Each core computes its local normalized
contexts, the tiny [19,257] payloads are AllGather-ed across the 8
cores and summed locally (Pearson is invariant to the 1/B scale), and
every core redundantly computes the replicated [19,19] correlation.

Per-core pipeline (all big work in bf16, fp32 PSUM accumulate):
  - both preds and feats arrive host-relayouted to spatial-major bf16,
    so the kernel is pure streaming matmul: no on-chip transposes, no
    dtype-cast DMAs, and half the HBM traffic of an fp32 layout.
  - feats carry an extra all-ones column per 128-position chunk, so the
    per-class mask sums fall out of the same matmuls (column 256 of the
    PSUM accumulator) instead of needing separate ones-matmuls.
  - feats stream over the SP HWDGE ring in ~2 MB slabs; the PE chases
    the slabs with [128,19]^T @ [128,257] accumulating matmuls.
  - small/latency-critical DMAs (collective bounce, output) ride the
    ACT HWDGE ring so they never queue behind the feats stream.
"""

import sys

sys.path.insert(0, "/opt/trn_rl_repo")

import numpy as np

import concourse.bass as bass  # noqa: F401  (import order matters)
import concourse.bacc as bacc
import concourse.tile as tile
import concourse.mybir as mybir
from concourse import bass_utils, bass2jax  # noqa: F401

B, K, C, H, W = 8, 19, 256, 128, 128
P = H * W            # 16384 spatial positions
NCHUNK = P // 128    # 128 contraction chunks
CCW = C + 1          # feats columns per chunk: 256 features + ones column
SLABC = 16           # chunks per feats DMA slab (~2.1 MB bf16)
N_CORES = 8

F32 = mybir.dt.float32
BF16 = mybir.dt.bfloat16


def _slab_schedule(nchunk):
    """Per-stream list of (chunk0, nchunks) DMA slabs.

    Stream 1's tail is split into smaller slabs so the final
    DMA-then-matmul dependency at the very end of the kernel is short.
    """
    slabs = []
    for s in (0, 1):
        sl, i = [], 0
        while i < nchunk:
            w = min(SLABC, nchunk - i)
            if s == 1 and i + w == nchunk and w == SLABC:
                sl.append((i, w // 2))
                sl.append((i + w // 2, w // 2))
            else:
                sl.append((i, w))
            i += w
        slabs.append(sl)
    return slabs


def build_body(nc, tc, pret_d, featsr_d, identf_d, out_d, n_cores,
               nchunk=NCHUNK):
    """Emit the per-core program.

    pret_d:   2 DRAM APs [128, nchunk*K] bf16 (preds, spatial-major)
    featsr_d: 2 DRAM APs [128, nchunk*CCW] bf16 (feats, spatial-major,
              with a ones column appended per chunk)
    identf_d: [K, K] fp32 identity (for PE transposes)
    out_d:    [K, K] fp32 output
    """
    mult = mybir.AluOpType.mult
    RG = [list(range(n_cores))]
    slabs = _slab_schedule(nchunk)
    last_chunk = nchunk - 1

    with tc.tile_pool(name="persist", bufs=1) as PP, \
         tc.tile_pool(name="acc0", bufs=1, space="PSUM") as PA0, \
         tc.tile_pool(name="acc1", bufs=1, space="PSUM") as PA1, \
         tc.tile_pool(name="tailp", bufs=1, space="PSUM") as TLP, \
         tc.tile_pool(name="dram", bufs=1, space="DRAM") as DP:

        # --- constants (ACT ring / DVE so the SP ring stays feats-only) ---
        id_f = PP.tile([K, K], F32, name="id_f")
        nc.scalar.dma_start(id_f[:], identf_d[:])
        ones19 = PP.tile([K, 1], F32, name="ones19")
        nc.vector.memset(ones19[:], 1.0)
        onesrow = PP.tile([1, K], F32, name="onesrow")
        nc.vector.memset(onesrow[:], 1.0)

        # --- preds (spatial-major bf16): stream 0's load leads the SP
        # ring; stream 1's is issued early in stream 0's slab train so it
        # lands well before stream 1's first matmul. ---
        PT = [PP.tile([128, nchunk * K], BF16, name=f"PT{s}") for s in (0, 1)]
        nc.sync.dma_start(PT[0][:], pret_d[0][:])

        psum_vec = [PA0.tile([K, CCW], F32, name="pvec0"),
                    PA1.tile([K, CCW], F32, name="pvec1")]
        nT = []
        prev_cc = None

        with tc.tile_pool(name="fslab", bufs=4) as FP:
            for s in (0, 1):
                for si, (c0, w) in enumerate(slabs[s]):
                    fsl = FP.tile([128, SLABC * CCW], BF16, name="fsl")
                    nc.sync.dma_start(fsl[:, 0:w * CCW],
                                      featsr_d[s][:, c0 * CCW:(c0 + w) * CCW])
                    if s == 0 and si == 1:
                        nc.sync.dma_start(PT[1][:], pret_d[1][:])
                    for t in range(w):
                        i = c0 + t
                        nc.tensor.matmul(
                            psum_vec[s][:],
                            lhsT=PT[s][:, i * K:(i + 1) * K],
                            rhs=fsl[:, t * CCW:(t + 1) * CCW],
                            start=(i == 0), stop=(i == last_chunk))

                # ---- stream epilogue (stream 0's overlaps stream 1) ----
                # mask sums arrived in PSUM column 256 via the ones column
                recip = PP.tile([K, 1], F32, name=f"recip{s}")
                nc.vector.reciprocal(recip[:], psum_vec[s][:, C:C + 1])
                vec_sb = PP.tile([K, C], F32, name=f"vec_sb{s}")
                nc.vector.tensor_scalar_mul(vec_sb[:], psum_vec[s][:, 0:C],
                                            recip[:])
                sq = PP.tile([K, C], F32, name=f"sq{s}")
                nc.scalar.square(sq[:], vec_sb[:])
                # column sums over K via fp32 matmul with a ones vector
                pn = TLP.tile([1, C], F32, name="pn", tag="tlp")
                nc.tensor.matmul(pn[:], lhsT=ones19[:], rhs=sq[:],
                                 start=True, stop=True)
                # reference clamps the norm at 1e-12; the norm here is
                # O(1e-2) for non-degenerate input, so the clamp is a no-op.
                nsb = PP.tile([1, C], F32, name=f"nsb{s}")
                nc.scalar.sqrt(nsb[:], pn[:])
                rn = PP.tile([1, C], F32, name=f"rn{s}")
                nc.vector.reciprocal(rn[:], nsb[:])
                # broadcast 1/norm to the K partitions (rank-1 matmul)
                bc = TLP.tile([K, C], F32, name="bc", tag="tlp")
                nc.tensor.matmul(bc[:], lhsT=onesrow[:], rhs=rn[:],
                                 start=True, stop=True)
                cc_in = PP.tile([K, CCW], F32, name=f"cc_in{s}")
                nc.vector.tensor_mul(cc_in[:, 0:C], vec_sb[:], bc[:])
                # ship the per-core row-mean in the payload (mean over B and
                # mean over C commute)
                xdum = PP.tile([K, C], F32, name=f"xdum{s}")
                nc.scalar.activation(xdum[:], cc_in[:, 0:C],
                                     mybir.ActivationFunctionType.Copy,
                                     scale=1.0 / C,
                                     accum_out=cc_in[:, C:C + 1])

                # ---- AllGather the tiny payload, sum the 8 ranks locally
                # (cheaper than AllReduce at this size: 4.6 us vs 9.7 us
                # floor).  Stream 0's collective hides under stream 1's
                # compute; only stream 1's sits on the tail. ----
                b_in = DP.tile([K, CCW], F32, name=f"b_in{s}")
                b_out = DP.tile([n_cores * K, CCW], F32, name=f"b_out{s}")
                nc.scalar.dma_start(b_in[:], cc_in[:])
                cc = nc.gpsimd.collective_compute(
                    "AllGather", mybir.AluOpType.bypass,
                    replica_groups=RG,
                    ins=[b_in.opt()], outs=[b_out.opt()])
                if prev_cc is not None:
                    bass._add_dep_helper(
                        cc.ins, prev_cc.ins, sync=False,
                        reason="collectives in stream order")
                prev_cc = cc
                csb = PP.tile([K, n_cores * CCW], F32, name=f"csb{s}")
                nc.scalar.dma_start(
                    csb[:].rearrange("p (r c) -> p r c", r=n_cores),
                    b_out[:].rearrange("(r p) c -> p r c", r=n_cores))
                X = PP.tile([K, CCW], F32, name=f"X{s}")
                nc.vector.reduce_sum(
                    X[:], csb[:].rearrange("p (r c) -> p c r", r=n_cores),
                    axis=mybir.AxisListType.X)

                # ---- side-s Pearson prep (side 0 runs during stream 1;
                # only side 1 trails the last collective) ----
                ms = X[:, C:C + 1]
                xc = PP.tile([K, C], F32, name=f"xc{s}")
                nc.vector.tensor_scalar_sub(xc[:], X[:, 0:C], ms)
                xsq = PP.tile([K, C], F32, name=f"xsq{s}")
                ss = PP.tile([K, 1], F32, name=f"ss{s}")
                nc.scalar.activation(xsq[:], xc[:],
                                     mybir.ActivationFunctionType.Square,
                                     accum_out=ss[:])
                sd = PP.tile([K, 1], F32, name=f"sd{s}")
                nc.scalar.sqrt(sd[:], ss[:])
                ri = PP.tile([K, 1], F32, name=f"ri{s}")
                nc.vector.reciprocal(ri[:], sd[:])
                xn = PP.tile([K, C], F32, name=f"xn{s}")
                nc.vector.tensor_scalar(xn[:], X[:, 0:C], ms, ri[:],
                                        op0=mybir.AluOpType.subtract,
                                        op1=mult)
                # transpose [K, C] -> [C, K] in two 128-wide blocks
                tps = TLP.tile([128, 2 * K], F32, name=f"tps{s}", tag="tlp")
                for h in (0, 1):
                    nc.tensor.matmul(
                        tps[:, h * K:(h + 1) * K],
                        lhsT=xn[:, h * 128:(h + 1) * 128],
                        rhs=id_f[:],
                        is_transpose=True,
                        start=(h == 0), stop=(h == 1))
                nTs = PP.tile([128, 2 * K], F32, name=f"nT{s}")
                nc.vector.tensor_copy(nTs[:], tps[:])
                nT.append(nTs)

            # ---- final correlation ----
            po = TLP.tile([K, K], F32, name="po", tag="tlp")
            for h in (0, 1):
                nc.tensor.matmul(po[:],
                                 lhsT=nT[0][:, h * K:(h + 1) * K],
                                 rhs=nT[1][:, h * K:(h + 1) * K],
                                 start=(h == 0), stop=(h == 1))
            osb = PP.tile([K, K], F32, name="osb")
            nc.vector.tensor_copy(osb[:], po[:])
            nc.scalar.dma_start(out_d[:], osb[:])


def build(n_cores=N_CORES, nchunk=NCHUNK):
    nc = bacc.Bacc("TRN2", target_bir_lowering=False, debug=False,
                   enable_asserts=False, num_devices=n_cores)
    pret_d = [nc.dram_tensor(f"pret{s}", [128, nchunk * K], BF16,
                             kind="ExternalInput").ap() for s in (1, 2)]
    featsr_d = [nc.dram_tensor(f"featsr{s}", [128, nchunk * CCW], BF16,
                               kind="ExternalInput").ap() for s in (1, 2)]
    identf_d = nc.dram_tensor("identf", [K, K], F32,
                              kind="ExternalInput").ap()
    out_d = nc.dram_tensor("out", [K, K], F32, kind="ExternalOutput").ap()
    with tile.TileContext(nc) as tc:
        build_body(nc, tc, pret_d, featsr_d, identf_d, out_d, n_cores,
                   nchunk=nchunk)
    nc.compile()
    return nc


_NC_CACHE = {}


def _get_nc():
    if "nc" not in _NC_CACHE:
        _NC_CACHE["nc"] = build(N_CORES)
    return _NC_CACHE["nc"]


class Runner:
    """Executes the compiled Bass program on the first `n_cores` jax
    devices via shard_map, with inputs pre-staged on the devices (the
    analog of the native path's input pre-load in run_neff) so all
    cores start the NEFF near-simultaneously."""

    def __init__(self, nc, n_cores):
        import jax
        from jax.experimental.shard_map import shard_map
        from jax.sharding import Mesh, PartitionSpec, NamedSharding

        bass2jax.install_neuronx_cc_hook()
        self.jax = jax
        self.nc = nc
        self.n_cores = n_cores
        assert nc.dbg_addr is None
        partition_name = (nc.partition_id_tensor.name
                          if nc.partition_id_tensor else None)
        in_names, out_names, out_avals = [], [], []
        for alloc in nc.m.functions[0].allocations:
            if not isinstance(alloc, mybir.MemoryLocationSet):
                continue
            name = alloc.memorylocations[0].name
            if alloc.kind == "ExternalInput":
                if name != partition_name:
                    in_names.append(name)
            elif alloc.kind == "ExternalOutput":
                shape = tuple(alloc.tensor_shape)
                dtype = mybir.dt.np(alloc.dtype)
                out_names.append(name)
                out_avals.append(jax.core.ShapedArray(shape, dtype))
        self.param_names = list(in_names)
        n_params = len(in_names)
        full_in_names = list(in_names) + list(out_names)
        if partition_name is not None:
            full_in_names.append(partition_name)
        full_in_names = tuple(full_in_names)
        donate = tuple(range(n_params, n_params + len(out_names)))
        self.out_names = out_names
        self.out_avals = out_avals

        def _body(*args):
            operands = list(args)
            if partition_name is not None:
                operands.append(bass2jax.partition_id_tensor())
            outs = bass2jax._bass_exec_p.bind(
                *operands,
                out_avals=tuple(out_avals),
                in_names=full_in_names,
                out_names=tuple(out_names),
                lowering_input_output_aliases=(),
                sim_require_finite=True,
                sim_require_nnan=True,
                nc=nc,
            )
            return tuple(outs)

        devices = jax.devices()[:n_cores]
        assert len(devices) == n_cores
        self.mesh = Mesh(np.asarray(devices), ("core",))
        in_specs = (PartitionSpec("core"),) * (n_params + len(out_names))
        out_specs = (PartitionSpec("core"),) * len(out_names)
        self.fn = jax.jit(
            shard_map(_body, mesh=self.mesh, in_specs=in_specs,
                      out_specs=out_specs, check_rep=False),
            donate_argnums=donate, keep_unused=True)
        self.sharding = NamedSharding(self.mesh, PartitionSpec("core"))

    def put(self, in_maps):
        concat = [
            np.concatenate([np.asarray(in_maps[c][n])
                            for c in range(self.n_cores)], axis=0)
            for n in self.param_names
        ]
        arrs = [self.jax.device_put(a, self.sharding) for a in concat]
        self.jax.block_until_ready(arrs)
        return arrs

    def zeros(self):
        zs = [self.jax.device_put(
            np.zeros((self.n_cores * a.shape[0], *a.shape[1:]), a.dtype),
            self.sharding) for a in self.out_avals]
        self.jax.block_until_ready(zs)
        return zs

    def exec(self, dev_in):
        outs = self.fn(*dev_in, *self.zeros())
        self.jax.block_until_ready(outs)
        return {
            name: np.asarray(outs[i]).reshape(
                self.n_cores, *self.out_avals[i].shape)
            for i, name in enumerate(self.out_names)
        }


def _get_runner():
    if "runner" not in _NC_CACHE:
        _NC_CACHE["runner"] = Runner(_get_nc(), N_CORES)
    return _NC_CACHE["runner"]


def make_in_maps(preds1, feats1, preds2, feats2):
    import ml_dtypes
    identf = np.eye(K, dtype=np.float32)
    # feats [B, C, H, W] -> [B, W(p), H(i), C] bf16 with a ones column per
    # chunk: FS[p, i*257 + c] = feats[c, i*128 + p]; FS[p, i*257 + 256] = 1
    fa = []
    for f in (feats1, feats2):
        a = np.empty((B, 128, NCHUNK, CCW), dtype=ml_dtypes.bfloat16)
        a[..., :C] = f.transpose(0, 3, 2, 1)
        a[..., C] = 1.0
        fa.append(a.reshape(B, 128, NCHUNK * CCW))
    # preds [B, K, H, W] -> [B, W(p), H(i), K]: chunk i's columns are
    # P^T[i*128:(i+1)*128, :] with the spatial index on partitions
    pt = [p.transpose(0, 3, 2, 1).astype(ml_dtypes.bfloat16).reshape(
              B, 128, NCHUNK * K) for p in (preds1, preds2)]
    in_maps = []
    for b in range(B):
        in_maps.append({
            "pret1": pt[0][b],
            "pret2": pt[1][b],
            "featsr1": fa[0][b],
            "featsr2": fa[1][b],
            "identf": identf,
        })
    return in_maps


def kernel(preds1, feats1, preds2, feats2):
    runner = _get_runner()
    in_maps = make_in_maps(preds1, feats1, preds2, feats2)
    dev_in = runner.put(in_maps)
    outs = runner.exec(dev_in)
    return np.asarray(outs["out"][0], dtype=np.float32)


# revision 5
# speedup vs baseline: 1.3638x; 1.3638x over previous
"""Trainium2 Bass kernel for nn_CategoryAlign_Module (pooling / cross Pearson).

Math (see reference):
  for each stream s in {1,2}:
    vec_b[k,c]  = sum_p preds[b,k,p] * feats[b,c,p] / sum_p preds[b,k,p]
    ctx_b[k,c]  = vec_b[k,c] / max(||vec_b[:,c]||_2, 1e-12)      (norm over K)
    ctx[k,c]    = mean_b ctx_b[k,c]
  out = pearson(ctx1, ctx2)   (center+normalize rows over C, then ctx1 @ ctx2^T)

Distribution: data-parallel over the batch dim, one batch element per
NeuronCore (B=8, 8 cores).  Each core computes its local normalized
contexts, the tiny [19,257] payloads are AllGather-ed across the 8
cores and summed locally (Pearson is invariant to the 1/B scale), and
every core redundantly computes the replicated [19,19] correlation.

Per-core pipeline (all big work in bf16, fp32 PSUM accumulate):
  - both preds and feats arrive host-relayouted to spatial-major bf16,
    so the kernel is pure streaming matmul: no on-chip transposes, no
    dtype-cast DMAs, and half the HBM traffic of an fp32 layout.
  - feats carry an extra all-ones column per 128-position chunk, so the
    per-class mask sums fall out of the same matmuls (column 256 of the
    PSUM accumulator) instead of needing separate ones-matmuls.
  - feats stream over the SP HWDGE ring in ~2 MB slabs; the PE chases
    the slabs with [128,19]^T @ [128,257] accumulating matmuls.
  - engine-queue discipline (queues are strict FIFO, so a DMA that
    waits on a collective must never be queued ahead of compute):
    SP ring carries the feats stream and, emitted after ALL slab
    issues, the post-collective readbacks + final output store; the
    ACT ring carries only payload bounce-out DMAs (which never wait
    on collectives); gpsimd carries the collective triggers.
  - the tail is minimized: stream 2's Pearson operand is transposed
    *centered only* (no wait on its 1/std), and the 1/std column scale
    is applied to the final [19,19] product via a rank-1 broadcast.
"""

import sys

sys.path.insert(0, "/opt/trn_rl_repo")

import numpy as np

import concourse.bass as bass  # noqa: F401  (import order matters)
import concourse.bacc as bacc
import concourse.tile as tile
import concourse.mybir as mybir
from concourse import bass_utils, bass2jax  # noqa: F401

B, K, C, H, W = 8, 19, 256, 128, 128
P = H * W            # 16384 spatial positions
NCHUNK = P // 128    # 128 contraction chunks
CCW = C + 1          # feats columns per chunk: 256 features + ones column
SLABC = 16           # chunks per feats DMA slab (~2.1 MB bf16)
N_CORES = 8

F32 = mybir.dt.float32
BF16 = mybir.dt.bfloat16


def _slab_schedule(nchunk):
    """Per-stream list of (chunk0, nchunks) DMA slabs.

    Stream 1's tail is split into small slabs so the final
    DMA-then-matmul dependency at the very end of the pipeline is
    short.
    """
    slabs = []
    for s in (0, 1):
        sl, i = [], 0
        while i < nchunk:
            w = min(SLABC, nchunk - i)
            if s == 1 and i + w == nchunk and w == SLABC:
                for w2 in (8, 4, 4):
                    sl.append((i, w2))
                    i += w2
            else:
                sl.append((i, w))
                i += w
        slabs.append(sl)
    return slabs


def build_body(nc, tc, pret_d, featsr_d, identf_d, out_d, n_cores,
               nchunk=NCHUNK):
    """Emit the per-core program.

    pret_d:   2 DRAM APs [128, nchunk*K] bf16 (preds, spatial-major)
    featsr_d: 2 DRAM APs [128, nchunk*CCW] bf16 (feats, spatial-major,
              with a ones column appended per chunk)
    identf_d: [K, K] fp32 identity (for PE transposes)
    out_d:    [K, K] fp32 output
    """
    mult = mybir.AluOpType.mult
    RG = [list(range(n_cores))]
    slabs = _slab_schedule(nchunk)
    last_chunk = nchunk - 1

    with tc.tile_pool(name="persist", bufs=1) as PP, \
         tc.tile_pool(name="acc0", bufs=1, space="PSUM") as PA0, \
         tc.tile_pool(name="acc1", bufs=1, space="PSUM") as PA1, \
         tc.tile_pool(name="tailp", bufs=1, space="PSUM") as TLP, \
         tc.tile_pool(name="dram", bufs=1, space="DRAM") as DP:

        # --- constants (ACT ring / DVE so the SP ring stays feats-only) ---
        id_f = PP.tile([K, K], F32, name="id_f")
        nc.scalar.dma_start(id_f[:], identf_d[:])
        ones19 = PP.tile([K, 1], F32, name="ones19")
        nc.vector.memset(ones19[:], 1.0)
        onesrow = PP.tile([1, K], F32, name="onesrow")
        nc.vector.memset(onesrow[:], 1.0)

        # --- preds (spatial-major bf16): stream 0's load leads the SP
        # ring; stream 1's is issued early in stream 0's slab train so it
        # lands well before stream 1's first matmul. ---
        PT = [PP.tile([128, nchunk * K], BF16, name=f"PT{s}") for s in (0, 1)]
        nc.sync.dma_start(PT[0][:], pret_d[0][:])

        psum_vec = [PA0.tile([K, CCW], F32, name="pvec0"),
                    PA1.tile([K, CCW], F32, name="pvec1")]
        bounce = []
        prev_cc = None

        with tc.tile_pool(name="fslab", bufs=4) as FP:
            for s in (0, 1):
                for si, (c0, w) in enumerate(slabs[s]):
                    fsl = FP.tile([128, SLABC * CCW], BF16, name="fsl")
                    nc.sync.dma_start(fsl[:, 0:w * CCW],
                                      featsr_d[s][:, c0 * CCW:(c0 + w) * CCW])
                    if s == 0 and si == 1:
                        nc.sync.dma_start(PT[1][:], pret_d[1][:])
                    for t in range(w):
                        i = c0 + t
                        nc.tensor.matmul(
                            psum_vec[s][:],
                            lhsT=PT[s][:, i * K:(i + 1) * K],
                            rhs=fsl[:, t * CCW:(t + 1) * CCW],
                            start=(i == 0), stop=(i == last_chunk))

                # ---- stream epilogue: local normalized context payload.
                # Mask sums arrived in PSUM column 256 via the ones column.
                recip = PP.tile([K, 1], F32, name=f"recip{s}")
                nc.vector.reciprocal(recip[:], psum_vec[s][:, C:C + 1])
                vec_sb = PP.tile([K, C], F32, name=f"vec_sb{s}")
                nc.vector.tensor_scalar_mul(vec_sb[:], psum_vec[s][:, 0:C],
                                            recip[:])
                sq = PP.tile([K, C], F32, name=f"sq{s}")
                nc.vector.tensor_mul(sq[:], vec_sb[:], vec_sb[:])
                # column sums over K via fp32 matmul with a ones vector
                pn = TLP.tile([1, C], F32, name="pn", tag="tlp")
                nc.tensor.matmul(pn[:], lhsT=ones19[:], rhs=sq[:],
                                 start=True, stop=True)
                # reference clamps the norm at 1e-12; the norm here is
                # O(1e-2) for non-degenerate input, so the clamp is a no-op.
                nsb = PP.tile([1, C], F32, name=f"nsb{s}")
                nc.scalar.sqrt(nsb[:], pn[:])
                rn = PP.tile([1, C], F32, name=f"rn{s}")
                nc.vector.reciprocal(rn[:], nsb[:])
                # broadcast 1/norm to the K partitions (rank-1 matmul)
                bc = TLP.tile([K, C], F32, name="bc", tag="tlp")
                nc.tensor.matmul(bc[:], lhsT=onesrow[:], rhs=rn[:],
                                 start=True, stop=True)
                cc_in = PP.tile([K, CCW], F32, name=f"cc_in{s}")
                nc.vector.tensor_mul(cc_in[:, 0:C], vec_sb[:], bc[:])
                # bounce the big payload half out early; the row-mean
                # column follows as a tiny second DMA
                b_in = DP.tile([K, CCW], F32, name=f"b_in{s}")
                b_out = DP.tile([n_cores * K, CCW], F32, name=f"b_out{s}")
                nc.scalar.dma_start(b_in[:, 0:C], cc_in[:, 0:C])
                # ship the per-core row-mean in the payload (mean over B
                # and mean over C commute)
                xdum = PP.tile([K, C], F32, name=f"xdum{s}")
                nc.scalar.activation(xdum[:], cc_in[:, 0:C],
                                     mybir.ActivationFunctionType.Copy,
                                     scale=1.0 / C,
                                     accum_out=cc_in[:, C:C + 1])
                nc.scalar.dma_start(b_in[:, C:C + 1], cc_in[:, C:C + 1])

                # ---- AllGather the tiny payload (cheaper than AllReduce:
                # ~5 us vs ~10 us floor); ranks are summed locally after.
                # Stream 0's collective hides under stream 1's compute. ----
                cc = nc.gpsimd.collective_compute(
                    "AllGather", mybir.AluOpType.bypass,
                    replica_groups=RG,
                    ins=[b_in.opt()], outs=[b_out.opt()])
                if prev_cc is not None:
                    bass._add_dep_helper(
                        cc.ins, prev_cc.ins, sync=False,
                        reason="collectives in stream order")
                prev_cc = cc
                bounce.append(b_out)

            # ---- post-collective work, emitted AFTER every slab DMA
            # issue (SP ring) and after both stream epilogues (DVE/ACT/PE
            # queues) so its collective-waits never head-of-line-block
            # the streaming phase. ----
            def readback_sum(s):
                """DMA the AllGather result back and sum the 8 rank
                blocks with contiguous halving adds."""
                csb = PP.tile([K, n_cores * CCW], F32, name=f"csb{s}")
                nc.sync.dma_start(
                    csb[:].rearrange("p (r c) -> p r c", r=n_cores),
                    bounce[s][:].rearrange("(r p) c -> p r c", r=n_cores))
                a1 = PP.tile([K, 4 * CCW], F32, name=f"a1_{s}")
                nc.vector.tensor_add(a1[:], csb[:, 0:4 * CCW],
                                     csb[:, 4 * CCW:8 * CCW])
                a2 = PP.tile([K, 2 * CCW], F32, name=f"a2_{s}")
                nc.vector.tensor_add(a2[:], a1[:, 0:2 * CCW],
                                     a1[:, 2 * CCW:4 * CCW])
                X = PP.tile([K, CCW], F32, name=f"X{s}")
                nc.vector.tensor_add(X[:], a2[:, 0:CCW], a2[:, CCW:2 * CCW])
                return X

            # side 0: full Pearson prep (hidden under stream 1 + AG1)
            X0 = readback_sum(0)
            ms0 = X0[:, C:C + 1]
            xc0 = PP.tile([K, C], F32, name="xc0")
            nc.vector.tensor_scalar_sub(xc0[:], X0[:, 0:C], ms0)
            xsq0 = PP.tile([K, C], F32, name="xsq0")
            ss0 = PP.tile([K, 1], F32, name="ss0")
            nc.scalar.activation(xsq0[:], xc0[:],
                                 mybir.ActivationFunctionType.Square,
                                 accum_out=ss0[:])
            sd0 = PP.tile([K, 1], F32, name="sd0")
            nc.scalar.sqrt(sd0[:], ss0[:])
            ri0 = PP.tile([K, 1], F32, name="ri0")
            nc.vector.reciprocal(ri0[:], sd0[:])
            xn0 = PP.tile([K, C], F32, name="xn0")
            nc.vector.tensor_scalar(xn0[:], X0[:, 0:C], ms0, ri0[:],
                                    op0=mybir.AluOpType.subtract, op1=mult)
            tps0 = TLP.tile([128, 2 * K], F32, name="tps0", tag="tlpA")
            for h in (0, 1):
                nc.tensor.matmul(tps0[:, h * K:(h + 1) * K],
                                 lhsT=xn0[:, h * 128:(h + 1) * 128],
                                 rhs=id_f[:], is_transpose=True,
                                 start=(h == 0), stop=(h == 1))
            nT0 = PP.tile([128, 2 * K], F32, name="nT0")
            nc.vector.tensor_copy(nT0[:], tps0[:])

            # side 1: minimal critical path after AG1.  Transpose the
            # CENTERED matrix (no dependency on 1/std), run the final
            # matmuls against the fully-normalized side 0, and apply
            # side 1's 1/std as a rank-1 column scale on the [19,19]
            # result.
            X1 = readback_sum(1)
            ms1 = X1[:, C:C + 1]
            xc1 = PP.tile([K, C], F32, name="xc1")
            nc.vector.tensor_scalar_sub(xc1[:], X1[:, 0:C], ms1)
            # 1/std chain on ACT/DVE, concurrent with the PE transposes
            xsq1 = PP.tile([K, C], F32, name="xsq1")
            ss1 = PP.tile([K, 1], F32, name="ss1")
            nc.scalar.activation(xsq1[:], xc1[:],
                                 mybir.ActivationFunctionType.Square,
                                 accum_out=ss1[:])
            sd1 = PP.tile([K, 1], F32, name="sd1")
            nc.scalar.sqrt(sd1[:], ss1[:])
            ri1 = PP.tile([K, 1], F32, name="ri1")
            nc.vector.reciprocal(ri1[:], sd1[:])
            tps1 = TLP.tile([128, 2 * K], F32, name="tps1", tag="tlpA")
            for h in (0, 1):
                nc.tensor.matmul(tps1[:, h * K:(h + 1) * K],
                                 lhsT=xc1[:, h * 128:(h + 1) * 128],
                                 rhs=id_f[:], is_transpose=True,
                                 start=(h == 0), stop=(h == 1))
            cT1 = PP.tile([128, 2 * K], F32, name="cT1")
            nc.vector.tensor_copy(cT1[:], tps1[:])
            # rank-1 broadcast of side-1's 1/std across columns
            riT = TLP.tile([1, K], F32, name="riT", tag="tlpB")
            nc.tensor.matmul(riT[:], lhsT=ri1[:], rhs=id_f[:],
                             is_transpose=True, start=True, stop=True)
            riTs = PP.tile([1, K], F32, name="riTs")
            nc.vector.tensor_copy(riTs[:], riT[:])
            rb = TLP.tile([K, K], F32, name="rb", tag="tlpB")
            nc.tensor.matmul(rb[:], lhsT=onesrow[:], rhs=riTs[:],
                             start=True, stop=True)
            rbs = PP.tile([K, K], F32, name="rbs")
            nc.vector.tensor_copy(rbs[:], rb[:])

            # ---- final correlation ----
            po = TLP.tile([K, K], F32, name="po", tag="tlpC")
            for h in (0, 1):
                nc.tensor.matmul(po[:],
                                 lhsT=nT0[:, h * K:(h + 1) * K],
                                 rhs=cT1[:, h * K:(h + 1) * K],
                                 start=(h == 0), stop=(h == 1))
            osb = PP.tile([K, K], F32, name="osb")
            nc.vector.tensor_mul(osb[:], po[:], rbs[:])
            nc.sync.dma_start(out_d[:], osb[:])


def build(n_cores=N_CORES, nchunk=NCHUNK):
    nc = bacc.Bacc("TRN2", target_bir_lowering=False, debug=False,
                   enable_asserts=False, num_devices=n_cores)
    pret_d = [nc.dram_tensor(f"pret{s}", [128, nchunk * K], BF16,
                             kind="ExternalInput").ap() for s in (1, 2)]
    featsr_d = [nc.dram_tensor(f"featsr{s}", [128, nchunk * CCW], BF16,
                               kind="ExternalInput").ap() for s in (1, 2)]
    identf_d = nc.dram_tensor("identf", [K, K], F32,
                              kind="ExternalInput").ap()
    out_d = nc.dram_tensor("out", [K, K], F32, kind="ExternalOutput").ap()
    with tile.TileContext(nc) as tc:
        build_body(nc, tc, pret_d, featsr_d, identf_d, out_d, n_cores,
                   nchunk=nchunk)
    nc.compile()
    return nc


_NC_CACHE = {}


def _get_nc():
    if "nc" not in _NC_CACHE:
        _NC_CACHE["nc"] = build(N_CORES)
    return _NC_CACHE["nc"]


class Runner:
    """Executes the compiled Bass program on the first `n_cores` jax
    devices via shard_map, with inputs pre-staged on the devices (the
    analog of the native path's input pre-load in run_neff) so all
    cores start the NEFF near-simultaneously."""

    def __init__(self, nc, n_cores):
        import jax
        from jax.experimental.shard_map import shard_map
        from jax.sharding import Mesh, PartitionSpec, NamedSharding

        bass2jax.install_neuronx_cc_hook()
        self.jax = jax
        self.nc = nc
        self.n_cores = n_cores
        assert nc.dbg_addr is None
        partition_name = (nc.partition_id_tensor.name
                          if nc.partition_id_tensor else None)
        in_names, out_names, out_avals = [], [], []
        for alloc in nc.m.functions[0].allocations:
            if not isinstance(alloc, mybir.MemoryLocationSet):
                continue
            name = alloc.memorylocations[0].name
            if alloc.kind == "ExternalInput":
                if name != partition_name:
                    in_names.append(name)
            elif alloc.kind == "ExternalOutput":
                shape = tuple(alloc.tensor_shape)
                dtype = mybir.dt.np(alloc.dtype)
                out_names.append(name)
                out_avals.append(jax.core.ShapedArray(shape, dtype))
        self.param_names = list(in_names)
        n_params = len(in_names)
        full_in_names = list(in_names) + list(out_names)
        if partition_name is not None:
            full_in_names.append(partition_name)
        full_in_names = tuple(full_in_names)
        donate = tuple(range(n_params, n_params + len(out_names)))
        self.out_names = out_names
        self.out_avals = out_avals

        def _body(*args):
            operands = list(args)
            if partition_name is not None:
                operands.append(bass2jax.partition_id_tensor())
            outs = bass2jax._bass_exec_p.bind(
                *operands,
                out_avals=tuple(out_avals),
                in_names=full_in_names,
                out_names=tuple(out_names),
                lowering_input_output_aliases=(),
                sim_require_finite=True,
                sim_require_nnan=True,
                nc=nc,
            )
            return tuple(outs)

        devices = jax.devices()[:n_cores]
        assert len(devices) == n_cores
        self.mesh = Mesh(np.asarray(devices), ("core",))
        in_specs = (PartitionSpec("core"),) * (n_params + len(out_names))
        out_specs = (PartitionSpec("core"),) * len(out_names)
        self.fn = jax.jit(
            shard_map(_body, mesh=self.mesh, in_specs=in_specs,
                      out_specs=out_specs, check_rep=False),
            donate_argnums=donate, keep_unused=True)
        self.sharding = NamedSharding(self.mesh, PartitionSpec("core"))

    def put(self, in_maps):
        concat = [
            np.concatenate([np.asarray(in_maps[c][n])
                            for c in range(self.n_cores)], axis=0)
            for n in self.param_names
        ]
        arrs = [self.jax.device_put(a, self.sharding) for a in concat]
        self.jax.block_until_ready(arrs)
        return arrs

    def zeros(self):
        zs = [self.jax.device_put(
            np.zeros((self.n_cores * a.shape[0], *a.shape[1:]), a.dtype),
            self.sharding) for a in self.out_avals]
        self.jax.block_until_ready(zs)
        return zs

    def exec(self, dev_in):
        outs = self.fn(*dev_in, *self.zeros())
        self.jax.block_until_ready(outs)
        return {
            name: np.asarray(outs[i]).reshape(
                self.n_cores, *self.out_avals[i].shape)
            for i, name in enumerate(self.out_names)
        }


def _get_runner():
    if "runner" not in _NC_CACHE:
        _NC_CACHE["runner"] = Runner(_get_nc(), N_CORES)
    return _NC_CACHE["runner"]


def make_in_maps(preds1, feats1, preds2, feats2):
    import ml_dtypes
    identf = np.eye(K, dtype=np.float32)
    # feats [B, C, H, W] -> [B, W(p), H(i), C] bf16 with a ones column per
    # chunk: FS[p, i*257 + c] = feats[c, i*128 + p]; FS[p, i*257 + 256] = 1
    fa = []
    for f in (feats1, feats2):
        a = np.empty((B, 128, NCHUNK, CCW), dtype=ml_dtypes.bfloat16)
        a[..., :C] = f.transpose(0, 3, 2, 1)
        a[..., C] = 1.0
        fa.append(a.reshape(B, 128, NCHUNK * CCW))
    # preds [B, K, H, W] -> [B, W(p), H(i), K]: chunk i's columns are
    # P^T[i*128:(i+1)*128, :] with the spatial index on partitions
    pt = [p.transpose(0, 3, 2, 1).astype(ml_dtypes.bfloat16).reshape(
              B, 128, NCHUNK * K) for p in (preds1, preds2)]
    in_maps = []
    for b in range(B):
        in_maps.append({
            "pret1": pt[0][b],
            "pret2": pt[1][b],
            "featsr1": fa[0][b],
            "featsr2": fa[1][b],
            "identf": identf,
        })
    return in_maps


def kernel(preds1, feats1, preds2, feats2):
    runner = _get_runner()
    in_maps = make_in_maps(preds1, feats1, preds2, feats2)
    dev_in = runner.put(in_maps)
    outs = runner.exec(dev_in)
    return np.asarray(outs["out"][0], dtype=np.float32)
